# revision 40
# baseline (speedup 1.0000x reference)
"""Mamba block (MockMambaBlock) on 8 Trainium2 NeuronCores.

Sharding: tensor-parallel over d_inner (8 x 256 channels), both batches
on every core. x_proj/dt partials complete via on-device AllReduce;
out_proj row-partials are summed on the host.

vs the original baseline (~647us -> ~535us): (1) the ring1 AllReduce of
each batch is split into two L/2 halves pipelined with phase A, and
dt_proj runs per-half as soon as its AR lands, so the first scan starts
~40us earlier; (2) the first two scan pairs run at half-L granularity
chained through the scan's initial operand, filling the wait for the
second AR half; (3) each pair's da=exp activation is emitted one chunk
ahead of the phase-A silus to dodge act-table reload stalls; (4) the
last pair of (1,1) is scanned in 4 chained 512-col chunks feeding
per-pt identity-sum -> gate -> out_proj, shrinking the tail.

Layout: channels on partitions, tokens along the free dim. The per-
(d,n) scan runs on DVE at ~2.1 cycles/col (dtype-independent; measured)
— Pool cannot execute TensorTensorScan, and any Pool activity (even
SWDGE DMAs) slows concurrent DVE scans via shared SBUF ports.
"""

import sys

sys.path.insert(0, "/opt/trn_rl_repo")

import numpy as np
import ml_dtypes

import concourse.bass as bass
import concourse.bacc as bacc
import concourse.mybir as mybir
import concourse.tile as tile
from concourse.bass_utils import run_bass_kernel_spmd

F32 = mybir.dt.float32
BF16 = mybir.dt.bfloat16
AF = mybir.ActivationFunctionType
OP = mybir.AluOpType

B, L, DM, DI, DS, DC = 2, 2048, 1024, 2048, 16, 4
NCORES = 8
DIL = DI // NCORES          # 256 channels per core
NBLK = DIL // 128           # 2 partition blocks of channels
KBLK = DM // 128            # 8 contraction blocks for in_proj
LTA = 512                   # phase A token chunk
LH = L // 2                 # AR half length
CUT = DS + 5                # ring1 rows: dt(16) + B states 0..4
POOL_U2 = False             # Pool u2 slows concurrent DVE scans (shared ports)
SCAN_DT = F32
Z_DT = BF16
XC_DT = BF16


def build_nc():
    nc = bacc.Bacc()

    x_t = nc.dram_tensor("x_t", [B, L // LTA, 128, KBLK, LTA], BF16,
                         kind="ExternalInput")
    win_d = nc.dram_tensor("win", [DM, 2 * DIL], BF16, kind="ExternalInput")
    wout_d = nc.dram_tensor("wout", [DIL, DM], BF16, kind="ExternalInput")
    wx_d = nc.dram_tensor("wx", [DIL, 2 * DS], BF16, kind="ExternalInput")
    wdt_d = nc.dram_tensor("wdt", [DS, DIL], BF16, kind="ExternalInput")
    a_d = nc.dram_tensor("a", [DIL, DS], F32, kind="ExternalInput")
    convw_d = nc.dram_tensor("convw", [DIL, DC], F32, kind="ExternalInput")
    convb_d = nc.dram_tensor("convb", [DIL, 1], F32, kind="ExternalInput")
    dvec_d = nc.dram_tensor("dvec", [DIL, 1], F32, kind="ExternalInput")
    bdt_d = nc.dram_tensor("bdt", [DIL, 1], F32, kind="ExternalInput")
    identb_d = nc.dram_tensor("identb", [128, 128], BF16, kind="ExternalInput")
    diagd_d = nc.dram_tensor("diagd", [DIL, 128], BF16, kind="ExternalInput")
    diagk_d = nc.dram_tensor("diagk", [DC, DIL, 128], BF16, kind="ExternalInput")
    out_d = nc.dram_tensor("out_p", [B, L, DM], BF16, kind="ExternalOutput")

    ncha = L // LTA

    with tile.TileContext(nc) as tc:
        with (
            tc.tile_pool(name="weights", bufs=1) as wp,
            tc.tile_pool(name="resident", bufs=1) as rp,
            tc.tile_pool(name="dram", bufs=1, space="DRAM") as dp,
        ):
            # ---- weights to SBUF ----
            win_sb = wp.tile([128, KBLK, 2 * DIL], BF16)
            nc.sync.dma_start(win_sb[:], win_d[:].rearrange("(k p) m -> p k m", p=128))
            wout_sb = wp.tile([128, NBLK, DM], BF16)
            nc.sync.dma_start(wout_sb[:], wout_d[:].rearrange("(k p) m -> p k m", p=128))
            wx_sb = wp.tile([128, NBLK, 2 * DS], BF16)
            nc.sync.dma_start(wx_sb[:], wx_d[:].rearrange("(k p) m -> p k m", p=128))
            wdt_sb = wp.tile([DS, DIL], BF16)
            nc.sync.dma_start(wdt_sb[:], wdt_d[:])
            a_sb = wp.tile([128, NBLK, DS], F32)
            nc.sync.dma_start(a_sb[:], a_d[:].rearrange("(k p) m -> p k m", p=128))
            convw_sb = wp.tile([128, NBLK, DC], F32)
            nc.sync.dma_start(convw_sb[:], convw_d[:].rearrange("(k p) m -> p k m", p=128))
            convb_sb = wp.tile([128, NBLK, 1], F32)
            nc.sync.dma_start(convb_sb[:], convb_d[:].rearrange("(k p) m -> p k m", p=128))
            dvec_sb = wp.tile([128, NBLK, 1], F32)
            nc.sync.dma_start(dvec_sb[:], dvec_d[:].rearrange("(k p) m -> p k m", p=128))
            bdt_sb = wp.tile([128, NBLK, 1], F32)
            nc.sync.dma_start(bdt_sb[:], bdt_d[:].rearrange("(k p) m -> p k m", p=128))
            identb_sb = wp.tile([128, 128], BF16)
            nc.sync.dma_start(identb_sb[:], identb_d[:])
            diagd_sb = wp.tile([128, NBLK, 128], BF16)
            nc.sync.dma_start(diagd_sb[:], diagd_d[:].rearrange("(k p) m -> p k m", p=128))
            diagk_sb = wp.tile([128, DC, NBLK, 128], BF16)
            nc.sync.dma_start(
                diagk_sb[:],
                diagk_d[:].rearrange("c (k p) m -> p c k m", p=128))

            # ---- resident activations ----
            xcv = [[rp.tile([128, L], XC_DT, name=f"xcv{b_}{k}", tag=f"xcv{b_}{k}")
                    for k in range(NBLK)] for b_ in range(B)]
            zac = [[rp.tile([128, L], Z_DT, name=f"zac{b_}{k}", tag=f"zac{b_}{k}")
                    for k in range(NBLK)] for b_ in range(B)]
            dtin_sb = [rp.tile([DS, L], BF16, name=f"dtin{b_}", tag=f"dtin{b_}")
                       for b_ in range(B)]
            md = [[rp.tile([128, L], BF16, name=f"md{b_}{k}", tag=f"md{b_}{k}")
                   for k in range(NBLK)] for b_ in range(B)]
            xp = [[rp.tile([128, L + DC - 1], BF16, name=f"xp{b_}{k}",
                           tag=f"xp{b_}{k}") for k in range(NBLK)]
                  for b_ in range(B)]
            yin = [[rp.tile([128, L], BF16, name=f"yin{b_}{k}",
                            tag=f"yin{b_}{k}")
                    for k in range(NBLK)] for b_ in range(B)]

            # AR staging: ring1 in two L/2 halves, ring2 whole-L
            cc_in1 = [[dp.tile([CUT, LH], BF16, name=f"cc_in1_{b_}{h}")
                       for h in range(2)] for b_ in range(B)]
            cc_in2 = [dp.tile([2 * DS - CUT, L], BF16, name=f"cc_in2_{b_}")
                      for b_ in range(B)]
            cc_out1 = [[dp.tile([CUT, LH], BF16, addr_space="Shared",
                                name=f"cc_out1_{b_}{h}") for h in range(2)]
                       for b_ in range(B)]
            cc_out2 = [dp.tile([2 * DS - CUT, L], BF16, addr_space="Shared",
                               name=f"cc_out2_{b_}") for b_ in range(B)]

            # PSUM budget (8 banks): ps_in(2) + cps(1) + ps_xs(1) +
            # y_ps(4) = 8. ps_dt / ps_o reuse ps_in.
            with (
                tc.tile_pool(name="pa", bufs=2) as pa,
                tc.tile_pool(name="pa_ps", bufs=2, space="PSUM") as paps,
                tc.tile_pool(name="pb", bufs=2) as pb,
                tc.tile_pool(name="pb_ps", bufs=1, space="PSUM") as pbps,
            ):
                def z_chunk(b_, ch):
                    t0 = ch * LTA
                    xs_all = pa.tile([128, KBLK, LTA], BF16, tag="xs_all",
                                     bufs=3)
                    nc.sync.dma_start(xs_all[:], x_t[b_, ch])
                    for m in range(NBLK, 2 * NBLK):
                        ps = paps.tile([128, LTA], F32, tag="ps_in", bufs=2)
                        for kb in range(KBLK):
                            nc.tensor.matmul(
                                ps[:],
                                win_sb[:, kb, m * 128:(m + 1) * 128],
                                xs_all[:, kb, :],
                                start=(kb == 0), stop=(kb == KBLK - 1))
                        blk = m - NBLK
                        nc.scalar.activation(
                            zac[b_][blk][:, t0:t0 + LTA], ps[:], AF.Silu)

                def phase_a_chunk(b_, ch, skip_z=False):
                    t0 = ch * LTA
                    xs_all = pa.tile([128, KBLK, LTA], BF16, tag="xs_all",
                                     bufs=3)
                    nc.sync.dma_start(xs_all[:], x_t[b_, ch])
                    for m in range(NBLK if skip_z else 2 * NBLK):
                        ps = paps.tile([128, LTA], F32, tag="ps_in", bufs=2)
                        for kb in range(KBLK):
                            nc.tensor.matmul(
                                ps[:],
                                win_sb[:, kb, m * 128:(m + 1) * 128],
                                xs_all[:, kb, :],
                                start=(kb == 0), stop=(kb == KBLK - 1))
                        if m < NBLK:
                            blk = m
                            if ch == 0:
                                nc.vector.memset(xp[b_][blk][:, 0:DC - 1], 0.0)
                            nc.scalar.copy(
                                xp[b_][blk][:, DC - 1 + t0:DC - 1 + t0 + LTA],
                                ps[:])
                            if b_ == 0:
                                cacc = pa.tile([128, LTA], F32, tag="cacc",
                                               bufs=1)
                                nc.vector.tensor_scalar_mul(
                                    cacc[:], xp[b_][blk][:, t0:t0 + LTA],
                                    convw_sb[:, blk, 0:1])
                                for k in range(1, DC):
                                    nc.vector.scalar_tensor_tensor(
                                        cacc[:],
                                        xp[b_][blk][:, t0 + k:t0 + k + LTA],
                                        convw_sb[:, blk, k:k + 1], cacc[:],
                                        OP.mult, OP.add)
                                nc.scalar.activation(
                                    xcv[b_][blk][:, t0:t0 + LTA], cacc[:],
                                    AF.Silu, bias=convb_sb[:, blk, :])
                            else:
                                cps = paps.tile([128, LTA], F32, tag="cps",
                                                bufs=1)
                                for k in range(DC):
                                    nc.tensor.matmul(
                                        cps[:],
                                        diagk_sb[:, k, blk, :],
                                        xp[b_][blk][:, t0 + k:t0 + k + LTA],
                                        start=(k == 0), stop=(k == DC - 1))
                                nc.scalar.activation(
                                    xcv[b_][blk][:, t0:t0 + LTA], cps[:],
                                    AF.Silu, bias=convb_sb[:, blk, :])
                        else:
                            blk = m - NBLK
                            nc.scalar.activation(
                                zac[b_][blk][:, t0:t0 + LTA], ps[:], AF.Silu)
                    ps_xs = paps.tile([2 * DS, LTA], F32, tag="ps_xs", bufs=1)
                    for kb in range(NBLK):
                        nc.tensor.matmul(
                            ps_xs[:],
                            wx_sb[:, kb, :],
                            xcv[b_][kb][:, t0:t0 + LTA],
                            start=(kb == 0), stop=(kb == NBLK - 1))
                    xs_sb = pa.tile([2 * DS, LTA], BF16, tag="xs_sb", bufs=2)
                    nc.scalar.copy(xs_sb[:], ps_xs[:])
                    h = t0 // LH
                    hoff = t0 - h * LH
                    nc.scalar.dma_start(
                        cc_in1[b_][h][:, hoff:hoff + LTA], xs_sb[0:CUT, :])
                    nc.scalar.dma_start(
                        cc_in2[b_][:, t0:t0 + LTA], xs_sb[CUT:2 * DS, :])

                def ar_ring1(b_, h):
                    nc.gpsimd.collective_compute(
                        "AllReduce", OP.add,
                        ins=[cc_in1[b_][h][:].opt()],
                        outs=[cc_out1[b_][h][:].opt()],
                        replica_groups=[list(range(NCORES))])

                def dtin_load(b_, h):
                    nc.scalar.dma_start(dtin_sb[b_][:, h * LH:(h + 1) * LH],
                                        cc_out1[b_][h][0:DS, :])

                def ar_ring2(b_):
                    nc.gpsimd.collective_compute(
                        "AllReduce", OP.add,
                        ins=[cc_in2[b_][:].opt()],
                        outs=[cc_out2[b_].opt()],
                        replica_groups=[list(range(NCORES))])

                def dt_half(b_, h):
                    LTD = 512
                    for blk in range(NBLK):
                        for ch in range(LH // LTD):
                            t0 = h * LH + ch * LTD
                            ps_dt = paps.tile([128, LTD], F32, tag="ps_in",
                                              bufs=2)
                            nc.tensor.matmul(
                                ps_dt[:], wdt_sb[:, blk * 128:(blk + 1) * 128],
                                dtin_sb[b_][:, t0:t0 + LTD],
                                start=True, stop=True)
                            nc.scalar.activation(
                                md[b_][blk][:, t0:t0 + LTD], ps_dt[:],
                                AF.Sigmoid, bias=bdt_sb[:, blk, :], scale=-1.0)
                    for blk in range(NBLK):
                        nc.scalar.activation(
                            md[b_][blk][:, h * LH:(h + 1) * LH],
                            md[b_][blk][:, h * LH:(h + 1) * LH], AF.Ln)

                def mk_y_ps(b_, blk):
                    y_ps = [pbps.tile([128, 512], F32, tag=f"y_ps{pt}", bufs=1,
                                      name=f"yps{b_}{blk}{pt}")
                            for pt in range(4)]
                    for pt in range(4):
                        nc.tensor.matmul(
                            y_ps[pt][:], diagd_sb[:, blk, :],
                            xcv[b_][blk][:, pt * 512:(pt + 1) * 512],
                            start=True, stop=False)
                    return y_ps

                def b_prologue(b_, blk):
                    dtx = pb.tile([128, 1, L], BF16, tag="dtx", bufs=2,
                                  name=f"dtx{b_}{blk}")
                    nc.vector.tensor_mul(dtx[:, 0, :], md[b_][blk][:],
                                         xcv[b_][blk][:])
                    return dtx, mk_y_ps(b_, blk)

                def b_first_pairs(b_, blk, dtx):
                    """Pairs (0,1) and (2,3) at half-L granularity: all four
                    half-0 scans run right after the first ring1 AR half
                    (pair-1's B rows are also in ring1), chained into half-1
                    via the scan's initial operand once the second AR lands."""
                    bbs = [load_bb(b_, blk, 2 * p) for p in range(2)]
                    u2s = [pb.tile([128, 2, L], BF16, tag="un", bufs=2,
                                   name=f"u{b_}{blk}{2 * p}") for p in range(2)]
                    hs = [pb.tile([128, L], BF16, tag="hn", bufs=4,
                                  name=f"h{b_}{blk}{s}") for s in range(4)]
                    for h in range(2):
                        sl = slice(h * LH, (h + 1) * LH)
                        nc.vector.tensor_mul(dtx[:, 0, sl],
                                             md[b_][blk][:, sl],
                                             xcv[b_][blk][:, sl])
                        for p in range(2):
                            nc.vector.tensor_mul(
                                u2s[p][:, :, sl],
                                dtx[:, 0:1, sl].broadcast_to([128, 2, LH]),
                                bbs[p][:, :, sl])
                            for s_ in range(2):
                                s = 2 * p + s_
                                da = pb.tile([128, L], F32, tag="dan", bufs=2,
                                             name=f"da{b_}{blk}{s}h{h}")
                                nc.scalar.activation(
                                    da[:, sl], md[b_][blk][:, sl], AF.Exp,
                                    scale=a_sb[:, blk, s:s + 1])
                                init = 0.0 if h == 0 else hs[s][:, LH - 1:LH]
                                nc.vector.tensor_tensor_scan(
                                    hs[s][:, sl], da[:, sl], u2s[p][:, s_, sl],
                                    init, OP.mult, OP.add)
                    return hs

                def load_bb(b_, blk, n):
                    """Broadcast B_ssm rows for states n, n+1 to 128 parts."""
                    bb2 = pb.tile([128, 2, L], BF16, tag="bbn", bufs=2,
                                  name=f"bb{b_}{blk}{n}")
                    for s in range(2):
                        ns = n + s
                        if ns <= 4:
                            for h in range(2):
                                nc.sync.dma_start(
                                    bb2[:, s, h * LH:(h + 1) * LH],
                                    cc_out1[b_][h][16 + ns:17 + ns, :]
                                    .broadcast_to([128, LH]))
                        else:
                            nc.sync.dma_start(
                                bb2[:, s, :],
                                cc_out2[b_][ns - 5:ns - 4, :]
                                .broadcast_to([128, L]))
                    return bb2

                def b_n_pair(b_, blk, n, dtx, y_ps=None, lasts=(None, None),
                             pool=False):
                    bb2 = load_bb(b_, blk, n)
                    u2 = pb.tile([128, 2, L], BF16, tag="un", bufs=2,
                                 name=f"u{b_}{blk}{n}")
                    eng = nc.gpsimd if pool else nc.vector
                    eng.tensor_mul(
                        u2[:], dtx[:, 0:1, :].broadcast_to([128, 2, L]),
                        bb2[:])
                    hs = []
                    for s in range(2):
                        da = pb.tile([128, L], F32, tag="dan", bufs=2,
                                     name=f"da{b_}{blk}{n + s}")
                        nc.scalar.activation(da[:], md[b_][blk][:], AF.Exp,
                                             scale=a_sb[:, blk,
                                                        n + s:n + s + 1])
                        h = pb.tile([128, L], BF16, tag="hn", bufs=4,
                                    name=f"h{b_}{blk}{n + s}")
                        nc.vector.tensor_tensor_scan(h[:], da[:], u2[:, s, :],
                                                     0.0, OP.mult, OP.add)
                        if y_ps is not None:
                            b_n_ids(y_ps, h, (n + s == DS - 1)
                                    if lasts[s] is None else lasts[s])
                        hs.append(h)
                    return hs

                def b_n_ids(y_ps, h, last):
                    for pt in range(4):
                        nc.tensor.matmul(
                            y_ps[pt][:], identb_sb[:],
                            h[:, pt * 512:(pt + 1) * 512],
                            start=False, stop=last)

                def b_last_pair_chunked(b_, blk, dtx, y_ps):
                    """Last pair (states 14,15) scanned in 4 chained 512-col
                    chunks; each chunk immediately feeds its pt's identity
                    sum, gate, and out_proj columns so the tail never waits
                    for the full-L scans."""
                    n = DS - 2
                    bb2 = load_bb(b_, blk, n)
                    u2 = pb.tile([128, 2, L], BF16, tag="un", bufs=2,
                                 name=f"u{b_}{blk}{n}")
                    nc.vector.tensor_mul(
                        u2[:], dtx[:, 0:1, :].broadcast_to([128, 2, L]),
                        bb2[:])
                    das, hs = [], []
                    for s in range(2):
                        da = pb.tile([128, L], F32, tag="dan", bufs=2,
                                     name=f"da{b_}{blk}{n + s}")
                        nc.scalar.activation(da[:], md[b_][blk][:], AF.Exp,
                                             scale=a_sb[:, blk,
                                                        n + s:n + s + 1])
                        das.append(da)
                        hs.append(pb.tile([128, L], BF16, tag="hn", bufs=4,
                                          name=f"h{b_}{blk}{n + s}"))
                    LQ = 512
                    for c in range(4):
                        sl = slice(c * LQ, (c + 1) * LQ)
                        for s in range(2):
                            init = 0.0 if c == 0 else hs[s][:, c * LQ - 1:c * LQ]
                            nc.vector.tensor_tensor_scan(
                                hs[s][:, sl], das[s][:, sl], u2[:, s, sl],
                                init, OP.mult, OP.add)
                            nc.tensor.matmul(
                                y_ps[c][:], identb_sb[:], hs[s][:, sl],
                                start=False, stop=(s == 1))
                        b_gate(b_, blk, y_ps, pts=[c])
                        out_proj(b_, mts=range(c * 4, c * 4 + 4), alt_tag=c)

                def b_gate(b_, blk, y_ps, pts=range(4)):
                    for pt in pts:
                        nc.vector.tensor_mul(
                            yin[b_][blk][:, pt * 512:(pt + 1) * 512],
                            y_ps[pt][:],
                            zac[b_][blk][:, pt * 512:(pt + 1) * 512])

                def b_block(b_, blk):
                    dtx, y_ps = b_prologue(b_, blk)
                    for i, n in enumerate(range(0, DS, 2)):
                        b_n_pair(b_, blk, n, dtx, y_ps=y_ps,
                                 pool=POOL_U2 and (i % 2 == 1))
                    b_gate(b_, blk, y_ps)
                    return y_ps

                def out_proj(b_, mts=None, alt_tag=None):
                    if mts is None:
                        mts = range(L // 128)
                    for mt in mts:
                        for dmh in range(2):
                            if alt_tag is not None and dmh == 1:
                                ps_o = pbps.tile([128, 512], F32,
                                                 tag=f"y_ps{alt_tag}")
                            else:
                                ps_o = paps.tile([128, 512], F32, tag="ps_in",
                                                 bufs=2)
                            for blk in range(NBLK):
                                nc.tensor.matmul(
                                    ps_o[:],
                                    yin[b_][blk][:, mt * 128:(mt + 1) * 128],
                                    wout_sb[:, blk, dmh * 512:(dmh + 1) * 512],
                                    start=(blk == 0), stop=(blk == NBLK - 1))
                            osb = pb.tile([128, 512], BF16, tag="osb", bufs=2)
                            nc.scalar.copy(osb[:], ps_o[:])
                            nc.sync.dma_start(
                                out_d[b_, mt * 128:(mt + 1) * 128,
                                      dmh * 512:(dmh + 1) * 512],
                                osb[:])

                # ---------------- schedule ----------------
                phase_a_chunk(0, 0, skip_z=True)
                phase_a_chunk(0, 1, skip_z=True)
                ar_ring1(0, 0)
                phase_a_chunk(0, 2, skip_z=True)
                phase_a_chunk(0, 3, skip_z=True)
                ar_ring1(0, 1)
                ar_ring2(0)
                dtin_load(0, 0)
                dt_half(0, 0)
                dtin_load(0, 1)
                dt_half(0, 1)
                y00 = mk_y_ps(0, 0)
                dtx00 = pb.tile([128, 1, L], BF16, tag="dtx", bufs=2,
                                name="dtx00")
                # pair 0 at half granularity; pairs 1..3 interleave with
                # phase A(1); ids deferred so PE never blocks on a scan
                hkeep = b_first_pairs(0, 0, dtx00)
                for ch in range(ncha):
                    for h in hkeep:
                        b_n_ids(y00, h, False)
                    hkeep = []
                    if ch < 2:
                        hkeep.extend(b_n_pair(0, 0, 2 * (ch + 2), dtx00))
                    phase_a_chunk(1, ch)
                    if ch == 1:
                        ar_ring1(1, 0)
                    if ch == 3:
                        ar_ring1(1, 1)
                        ar_ring2(1)
                for h in hkeep:
                    b_n_ids(y00, h, False)
                for ch in range(ncha):
                    z_chunk(0, ch)
                for n in range(8, DS, 2):
                    b_n_pair(0, 0, n, dtx00, y_ps=y00)
                b_gate(0, 0, y00)
                b_block(0, 1)
                dtin_load(1, 0)
                dt_half(1, 0)
                dtin_load(1, 1)
                dt_half(1, 1)
                out_proj(0)
                b_block(1, 0)
                # b(1,1): last pair chunked so gates/out_proj overlap scans
                dtx11, y11 = b_prologue(1, 1)
                for n in range(0, DS - 2, 2):
                    b_n_pair(1, 1, n, dtx11, y_ps=y11)
                b_last_pair_chunked(1, 1, dtx11, y11)

    nc.compile()
    return nc


_NC_CACHE = {}


def _get_nc():
    if "nc" not in _NC_CACHE:
        _NC_CACHE["nc"] = build_nc()
    return _NC_CACHE["nc"]


def make_in_maps(x, W_in, conv_w, conv_b, W_x, W_dt, b_dt, A_log, D, W_out):
    x = np.asarray(x, np.float32)
    W_in = np.asarray(W_in, np.float32)
    conv_w = np.asarray(conv_w, np.float32)
    conv_b = np.asarray(conv_b, np.float32)
    W_x = np.asarray(W_x, np.float32)
    W_dt = np.asarray(W_dt, np.float32)
    b_dt = np.asarray(b_dt, np.float32)
    A_log = np.asarray(A_log, np.float32)
    D = np.asarray(D, np.float32)
    W_out = np.asarray(W_out, np.float32)

    xt = np.ascontiguousarray(
        x.reshape(B, L // 512, 512, KBLK, 128).transpose(0, 1, 4, 3, 2)
    ).astype(ml_dtypes.bfloat16)
    A = np.exp(A_log)

    in_maps = []
    for c in range(NCORES):
        lo = c * DIL
        sl = slice(lo, lo + DIL)
        cw = conv_w[sl]
        diagk = np.zeros((DC, DIL, 128), np.float32)
        for k in range(DC):
            for blk in range(NBLK):
                diagk[k, blk * 128:(blk + 1) * 128, :] = np.diag(
                    cw[blk * 128:(blk + 1) * 128, k])
        in_maps.append({
            "x_t": xt,
            "diagk": diagk.astype(ml_dtypes.bfloat16),
            "win": np.ascontiguousarray(
                np.concatenate([W_in[:, sl], W_in[:, DI + lo:DI + lo + DIL]],
                               axis=1)).astype(ml_dtypes.bfloat16),
            "wout": np.ascontiguousarray(W_out[sl]).astype(ml_dtypes.bfloat16),
            "wx": np.ascontiguousarray(
                np.concatenate([W_x[sl, :DS], -W_x[sl, DS:]], axis=1)
            ).astype(ml_dtypes.bfloat16),
            "wdt": np.ascontiguousarray(W_dt[:, sl]).astype(ml_dtypes.bfloat16),
            "a": np.ascontiguousarray(A[sl]),
            "convw": np.ascontiguousarray(conv_w[sl]),
            "convb": np.ascontiguousarray(conv_b[sl, None]),
            "dvec": np.ascontiguousarray(D[sl, None]),
            "bdt": np.ascontiguousarray(-b_dt[sl, None]),
            "identb": np.eye(128, dtype=ml_dtypes.bfloat16),
            "diagd": np.stack([np.diag(D[lo + k * 128:lo + (k + 1) * 128])
                               for k in range(NBLK)]).reshape(DIL, 128)
                       .astype(ml_dtypes.bfloat16),
        })
    return in_maps


def kernel(**inputs):
    nc = _get_nc()
    in_maps = make_in_maps(**inputs)
    res = run_bass_kernel_spmd(nc, in_maps, list(range(NCORES)))
    out = np.zeros((B, L, DM), np.float32)
    for c in range(NCORES):
        out += res.results[c]["out_p"].astype(np.float32)
    return out


# revision 41
# speedup vs baseline: 1.0075x; 1.0075x over previous
"""Mamba block (MockMambaBlock) on 8 Trainium2 NeuronCores.

Sharding: tensor-parallel over d_inner (8 x 256 channels), both batches
on every core. x_proj/dt partials complete via on-device AllReduce;
out_proj row-partials are summed on the host.

vs the original baseline (~647us -> ~535us): (1) the ring1 AllReduce of
each batch is split into two L/2 halves pipelined with phase A, and
dt_proj runs per-half as soon as its AR lands, so the first scan starts
~40us earlier; (2) the first two scan pairs run at half-L granularity
chained through the scan's initial operand, filling the wait for the
second AR half; (3) each pair's da=exp activation is emitted one chunk
ahead of the phase-A silus to dodge act-table reload stalls; (4) the
last pair of (1,1) is scanned in 4 chained 512-col chunks feeding
per-pt identity-sum -> gate -> out_proj, shrinking the tail.

Layout: channels on partitions, tokens along the free dim. The per-
(d,n) scan runs on DVE at ~2.1 cycles/col (dtype-independent; measured)
— Pool cannot execute TensorTensorScan, and any Pool activity (even
SWDGE DMAs) slows concurrent DVE scans via shared SBUF ports.
"""

import sys

sys.path.insert(0, "/opt/trn_rl_repo")

import numpy as np
import ml_dtypes

import concourse.bass as bass
import concourse.bacc as bacc
import concourse.mybir as mybir
import concourse.tile as tile
from concourse.bass_utils import run_bass_kernel_spmd

F32 = mybir.dt.float32
BF16 = mybir.dt.bfloat16
AF = mybir.ActivationFunctionType
OP = mybir.AluOpType

B, L, DM, DI, DS, DC = 2, 2048, 1024, 2048, 16, 4
NCORES = 8
DIL = DI // NCORES          # 256 channels per core
NBLK = DIL // 128           # 2 partition blocks of channels
KBLK = DM // 128            # 8 contraction blocks for in_proj
LTA = 512                   # phase A token chunk
LH = L // 2                 # AR half length
CUT = DS + 5                # ring1 rows: dt(16) + B states 0..4
POOL_U2 = False             # Pool u2 slows concurrent DVE scans (shared ports)
SCAN_DT = F32
Z_DT = BF16
XC_DT = BF16


def build_nc():
    nc = bacc.Bacc()

    x_t = nc.dram_tensor("x_t", [B, L // LTA, 128, KBLK, LTA], BF16,
                         kind="ExternalInput")
    win_d = nc.dram_tensor("win", [DM, 2 * DIL], BF16, kind="ExternalInput")
    wout_d = nc.dram_tensor("wout", [DIL, DM], BF16, kind="ExternalInput")
    wx_d = nc.dram_tensor("wx", [DIL, 2 * DS], BF16, kind="ExternalInput")
    wdt_d = nc.dram_tensor("wdt", [DS, DIL], BF16, kind="ExternalInput")
    a_d = nc.dram_tensor("a", [DIL, DS], F32, kind="ExternalInput")
    convw_d = nc.dram_tensor("convw", [DIL, DC], F32, kind="ExternalInput")
    convb_d = nc.dram_tensor("convb", [DIL, 1], F32, kind="ExternalInput")
    dvec_d = nc.dram_tensor("dvec", [DIL, 1], F32, kind="ExternalInput")
    bdt_d = nc.dram_tensor("bdt", [DIL, 1], F32, kind="ExternalInput")
    identb_d = nc.dram_tensor("identb", [128, 128], BF16, kind="ExternalInput")
    diagd_d = nc.dram_tensor("diagd", [DIL, 128], BF16, kind="ExternalInput")
    diagk_d = nc.dram_tensor("diagk", [DC, DIL, 128], BF16, kind="ExternalInput")
    out_d = nc.dram_tensor("out_p", [B, L, DM], BF16, kind="ExternalOutput")

    ncha = L // LTA

    with tile.TileContext(nc) as tc:
        with (
            tc.tile_pool(name="weights", bufs=1) as wp,
            tc.tile_pool(name="resident", bufs=1) as rp,
            tc.tile_pool(name="dram", bufs=1, space="DRAM") as dp,
        ):
            # ---- weights to SBUF ----
            win_sb = wp.tile([128, KBLK, 2 * DIL], BF16)
            nc.sync.dma_start(win_sb[:], win_d[:].rearrange("(k p) m -> p k m", p=128))
            wout_sb = wp.tile([128, NBLK, DM], BF16)
            nc.sync.dma_start(wout_sb[:], wout_d[:].rearrange("(k p) m -> p k m", p=128))
            wx_sb = wp.tile([128, NBLK, 2 * DS], BF16)
            nc.sync.dma_start(wx_sb[:], wx_d[:].rearrange("(k p) m -> p k m", p=128))
            wdt_sb = wp.tile([DS, DIL], BF16)
            nc.sync.dma_start(wdt_sb[:], wdt_d[:])
            a_sb = wp.tile([128, NBLK, DS], F32)
            nc.sync.dma_start(a_sb[:], a_d[:].rearrange("(k p) m -> p k m", p=128))
            convw_sb = wp.tile([128, NBLK, DC], F32)
            nc.sync.dma_start(convw_sb[:], convw_d[:].rearrange("(k p) m -> p k m", p=128))
            convb_sb = wp.tile([128, NBLK, 1], F32)
            nc.sync.dma_start(convb_sb[:], convb_d[:].rearrange("(k p) m -> p k m", p=128))
            dvec_sb = wp.tile([128, NBLK, 1], F32)
            nc.sync.dma_start(dvec_sb[:], dvec_d[:].rearrange("(k p) m -> p k m", p=128))
            bdt_sb = wp.tile([128, NBLK, 1], F32)
            nc.sync.dma_start(bdt_sb[:], bdt_d[:].rearrange("(k p) m -> p k m", p=128))
            identb_sb = wp.tile([128, 128], BF16)
            nc.sync.dma_start(identb_sb[:], identb_d[:])
            diagd_sb = wp.tile([128, NBLK, 128], BF16)
            nc.sync.dma_start(diagd_sb[:], diagd_d[:].rearrange("(k p) m -> p k m", p=128))
            diagk_sb = wp.tile([128, DC, NBLK, 128], BF16)
            nc.sync.dma_start(
                diagk_sb[:],
                diagk_d[:].rearrange("c (k p) m -> p c k m", p=128))

            # ---- resident activations ----
            xcv = [[rp.tile([128, L], XC_DT, name=f"xcv{b_}{k}", tag=f"xcv{b_}{k}")
                    for k in range(NBLK)] for b_ in range(B)]
            zac = [[rp.tile([128, L], Z_DT, name=f"zac{b_}{k}", tag=f"zac{b_}{k}")
                    for k in range(NBLK)] for b_ in range(B)]
            dtin_sb = [rp.tile([DS, L], BF16, name=f"dtin{b_}", tag=f"dtin{b_}")
                       for b_ in range(B)]
            md = [[rp.tile([128, L], BF16, name=f"md{b_}{k}", tag=f"md{b_}{k}")
                   for k in range(NBLK)] for b_ in range(B)]
            xp = [[rp.tile([128, L + DC - 1], BF16, name=f"xp{b_}{k}",
                           tag=f"xp{b_}{k}") for k in range(NBLK)]
                  for b_ in range(B)]
            yin = [[rp.tile([128, L], BF16, name=f"yin{b_}{k}",
                            tag=f"yin{b_}{k}")
                    for k in range(NBLK)] for b_ in range(B)]

            # AR staging: ring1 in two L/2 halves, ring2 whole-L
            cc_in1 = [[dp.tile([CUT, LH], BF16, name=f"cc_in1_{b_}{h}")
                       for h in range(2)] for b_ in range(B)]
            cc_in2 = [dp.tile([2 * DS - CUT, L], BF16, name=f"cc_in2_{b_}")
                      for b_ in range(B)]
            cc_out1 = [[dp.tile([CUT, LH], BF16, addr_space="Shared",
                                name=f"cc_out1_{b_}{h}") for h in range(2)]
                       for b_ in range(B)]
            cc_out2 = [dp.tile([2 * DS - CUT, L], BF16, addr_space="Shared",
                               name=f"cc_out2_{b_}") for b_ in range(B)]

            # PSUM budget (8 banks): ps_in(2) + cps(1) + ps_xs(1) +
            # y_ps(4) = 8. ps_dt / ps_o reuse ps_in.
            with (
                tc.tile_pool(name="pa", bufs=2) as pa,
                tc.tile_pool(name="pa_ps", bufs=2, space="PSUM") as paps,
                tc.tile_pool(name="pb", bufs=2) as pb,
                tc.tile_pool(name="pb_ps", bufs=1, space="PSUM") as pbps,
            ):
                def z_chunk(b_, ch):
                    t0 = ch * LTA
                    xs_all = pa.tile([128, KBLK, LTA], BF16, tag="xs_all",
                                     bufs=3)
                    nc.sync.dma_start(xs_all[:], x_t[b_, ch])
                    for m in range(NBLK, 2 * NBLK):
                        ps = paps.tile([128, LTA], F32, tag="ps_in", bufs=2)
                        for kb in range(KBLK):
                            nc.tensor.matmul(
                                ps[:],
                                win_sb[:, kb, m * 128:(m + 1) * 128],
                                xs_all[:, kb, :],
                                start=(kb == 0), stop=(kb == KBLK - 1))
                        blk = m - NBLK
                        nc.scalar.activation(
                            zac[b_][blk][:, t0:t0 + LTA], ps[:], AF.Silu)

                def phase_a_chunk(b_, ch, skip_z=False):
                    t0 = ch * LTA
                    xs_all = pa.tile([128, KBLK, LTA], BF16, tag="xs_all",
                                     bufs=3)
                    nc.sync.dma_start(xs_all[:], x_t[b_, ch])
                    for m in range(NBLK if skip_z else 2 * NBLK):
                        ps = paps.tile([128, LTA], F32, tag="ps_in", bufs=2)
                        for kb in range(KBLK):
                            nc.tensor.matmul(
                                ps[:],
                                win_sb[:, kb, m * 128:(m + 1) * 128],
                                xs_all[:, kb, :],
                                start=(kb == 0), stop=(kb == KBLK - 1))
                        if m < NBLK:
                            blk = m
                            if ch == 0:
                                nc.vector.memset(xp[b_][blk][:, 0:DC - 1], 0.0)
                            nc.scalar.copy(
                                xp[b_][blk][:, DC - 1 + t0:DC - 1 + t0 + LTA],
                                ps[:])
                            if b_ == 0:
                                cacc = pa.tile([128, LTA], F32, tag="cacc",
                                               bufs=1)
                                nc.vector.tensor_scalar_mul(
                                    cacc[:], xp[b_][blk][:, t0:t0 + LTA],
                                    convw_sb[:, blk, 0:1])
                                for k in range(1, DC):
                                    nc.vector.scalar_tensor_tensor(
                                        cacc[:],
                                        xp[b_][blk][:, t0 + k:t0 + k + LTA],
                                        convw_sb[:, blk, k:k + 1], cacc[:],
                                        OP.mult, OP.add)
                                nc.scalar.activation(
                                    xcv[b_][blk][:, t0:t0 + LTA], cacc[:],
                                    AF.Silu, bias=convb_sb[:, blk, :])
                            else:
                                cps = paps.tile([128, LTA], F32, tag="cps",
                                                bufs=1)
                                for k in range(DC):
                                    nc.tensor.matmul(
                                        cps[:],
                                        diagk_sb[:, k, blk, :],
                                        xp[b_][blk][:, t0 + k:t0 + k + LTA],
                                        start=(k == 0), stop=(k == DC - 1))
                                nc.scalar.activation(
                                    xcv[b_][blk][:, t0:t0 + LTA], cps[:],
                                    AF.Silu, bias=convb_sb[:, blk, :])
                        else:
                            blk = m - NBLK
                            nc.scalar.activation(
                                zac[b_][blk][:, t0:t0 + LTA], ps[:], AF.Silu)
                    ps_xs = paps.tile([2 * DS, LTA], F32, tag="ps_xs", bufs=1)
                    for kb in range(NBLK):
                        nc.tensor.matmul(
                            ps_xs[:],
                            wx_sb[:, kb, :],
                            xcv[b_][kb][:, t0:t0 + LTA],
                            start=(kb == 0), stop=(kb == NBLK - 1))
                    xs_sb = pa.tile([2 * DS, LTA], BF16, tag="xs_sb", bufs=2)
                    nc.scalar.copy(xs_sb[:], ps_xs[:])
                    h = t0 // LH
                    hoff = t0 - h * LH
                    nc.scalar.dma_start(
                        cc_in1[b_][h][:, hoff:hoff + LTA], xs_sb[0:CUT, :])
                    nc.scalar.dma_start(
                        cc_in2[b_][:, t0:t0 + LTA], xs_sb[CUT:2 * DS, :])

                def ar_ring1(b_, h):
                    nc.gpsimd.collective_compute(
                        "AllReduce", OP.add,
                        ins=[cc_in1[b_][h][:].opt()],
                        outs=[cc_out1[b_][h][:].opt()],
                        replica_groups=[list(range(NCORES))])

                def dtin_load(b_, h):
                    nc.scalar.dma_start(dtin_sb[b_][:, h * LH:(h + 1) * LH],
                                        cc_out1[b_][h][0:DS, :])

                def ar_ring2(b_):
                    nc.gpsimd.collective_compute(
                        "AllReduce", OP.add,
                        ins=[cc_in2[b_][:].opt()],
                        outs=[cc_out2[b_].opt()],
                        replica_groups=[list(range(NCORES))])

                def dt_half(b_, h):
                    LTD = 512
                    for blk in range(NBLK):
                        for ch in range(LH // LTD):
                            t0 = h * LH + ch * LTD
                            ps_dt = paps.tile([128, LTD], F32, tag="ps_in",
                                              bufs=2)
                            nc.tensor.matmul(
                                ps_dt[:], wdt_sb[:, blk * 128:(blk + 1) * 128],
                                dtin_sb[b_][:, t0:t0 + LTD],
                                start=True, stop=True)
                            nc.scalar.activation(
                                md[b_][blk][:, t0:t0 + LTD], ps_dt[:],
                                AF.Sigmoid, bias=bdt_sb[:, blk, :], scale=-1.0)
                    for blk in range(NBLK):
                        nc.scalar.activation(
                            md[b_][blk][:, h * LH:(h + 1) * LH],
                            md[b_][blk][:, h * LH:(h + 1) * LH], AF.Ln)

                def mk_y_ps(b_, blk):
                    y_ps = [pbps.tile([128, 512], F32, tag=f"y_ps{pt}", bufs=1,
                                      name=f"yps{b_}{blk}{pt}")
                            for pt in range(4)]
                    for pt in range(4):
                        nc.tensor.matmul(
                            y_ps[pt][:], diagd_sb[:, blk, :],
                            xcv[b_][blk][:, pt * 512:(pt + 1) * 512],
                            start=True, stop=False)
                    return y_ps

                def b_prologue(b_, blk):
                    dtx = pb.tile([128, 1, L], BF16, tag="dtx", bufs=2,
                                  name=f"dtx{b_}{blk}")
                    nc.vector.tensor_mul(dtx[:, 0, :], md[b_][blk][:],
                                         xcv[b_][blk][:])
                    return dtx, mk_y_ps(b_, blk)

                def b_first_pairs(b_, blk, dtx):
                    """Pairs (0,1) and (2,3) at half-L granularity: all four
                    half-0 scans run right after the first ring1 AR half
                    (pair-1's B rows are also in ring1), chained into half-1
                    via the scan's initial operand once the second AR lands."""
                    bbs = [load_bb(b_, blk, 2 * p) for p in range(2)]
                    u2s = [pb.tile([128, 2, L], BF16, tag="un", bufs=2,
                                   name=f"u{b_}{blk}{2 * p}") for p in range(2)]
                    hs = [pb.tile([128, L], BF16, tag="hn", bufs=4,
                                  name=f"h{b_}{blk}{s}") for s in range(4)]
                    for h in range(2):
                        sl = slice(h * LH, (h + 1) * LH)
                        nc.vector.tensor_mul(dtx[:, 0, sl],
                                             md[b_][blk][:, sl],
                                             xcv[b_][blk][:, sl])
                        for p in range(2):
                            nc.vector.tensor_mul(
                                u2s[p][:, :, sl],
                                dtx[:, 0:1, sl].broadcast_to([128, 2, LH]),
                                bbs[p][:, :, sl])
                            for s_ in range(2):
                                s = 2 * p + s_
                                da = pb.tile([128, L], F32, tag="dan", bufs=2,
                                             name=f"da{b_}{blk}{s}h{h}")
                                nc.scalar.activation(
                                    da[:, sl], md[b_][blk][:, sl], AF.Exp,
                                    scale=a_sb[:, blk, s:s + 1])
                                init = 0.0 if h == 0 else hs[s][:, LH - 1:LH]
                                nc.vector.tensor_tensor_scan(
                                    hs[s][:, sl], da[:, sl], u2s[p][:, s_, sl],
                                    init, OP.mult, OP.add)
                    return hs

                def load_bb(b_, blk, n):
                    """Broadcast B_ssm rows for states n, n+1 to 128 parts."""
                    bb2 = pb.tile([128, 2, L], BF16, tag="bbn", bufs=2,
                                  name=f"bb{b_}{blk}{n}")
                    for s in range(2):
                        ns = n + s
                        if ns <= 4:
                            for h in range(2):
                                nc.sync.dma_start(
                                    bb2[:, s, h * LH:(h + 1) * LH],
                                    cc_out1[b_][h][16 + ns:17 + ns, :]
                                    .broadcast_to([128, LH]))
                        else:
                            nc.sync.dma_start(
                                bb2[:, s, :],
                                cc_out2[b_][ns - 5:ns - 4, :]
                                .broadcast_to([128, L]))
                    return bb2

                def b_n_pair(b_, blk, n, dtx, y_ps=None, lasts=(None, None),
                             pool=False):
                    bb2 = load_bb(b_, blk, n)
                    u2 = pb.tile([128, 2, L], BF16, tag="un", bufs=2,
                                 name=f"u{b_}{blk}{n}")
                    eng = nc.gpsimd if pool else nc.vector
                    eng.tensor_mul(
                        u2[:], dtx[:, 0:1, :].broadcast_to([128, 2, L]),
                        bb2[:])
                    hs = []
                    for s in range(2):
                        da = pb.tile([128, L], F32, tag="dan", bufs=2,
                                     name=f"da{b_}{blk}{n + s}")
                        nc.scalar.activation(da[:], md[b_][blk][:], AF.Exp,
                                             scale=a_sb[:, blk,
                                                        n + s:n + s + 1])
                        h = pb.tile([128, L], BF16, tag="hn", bufs=4,
                                    name=f"h{b_}{blk}{n + s}")
                        nc.vector.tensor_tensor_scan(h[:], da[:], u2[:, s, :],
                                                     0.0, OP.mult, OP.add)
                        if y_ps is not None:
                            b_n_ids(y_ps, h, (n + s == DS - 1)
                                    if lasts[s] is None else lasts[s])
                        hs.append(h)
                    return hs

                def b_n_ids(y_ps, h, last):
                    for pt in range(4):
                        nc.tensor.matmul(
                            y_ps[pt][:], identb_sb[:],
                            h[:, pt * 512:(pt + 1) * 512],
                            start=False, stop=last)

                def b_last_pair_chunked(b_, blk, dtx, y_ps):
                    """Last pair (states 14,15) scanned in 4 chained 512-col
                    chunks; each chunk immediately feeds its pt's identity
                    sum, gate, and out_proj columns so the tail never waits
                    for the full-L scans."""
                    n = DS - 2
                    bb2 = load_bb(b_, blk, n)
                    u2 = pb.tile([128, 2, L], BF16, tag="un", bufs=2,
                                 name=f"u{b_}{blk}{n}")
                    nc.vector.tensor_mul(
                        u2[:], dtx[:, 0:1, :].broadcast_to([128, 2, L]),
                        bb2[:])
                    das, hs = [], []
                    for s in range(2):
                        da = pb.tile([128, L], F32, tag="dan", bufs=2,
                                     name=f"da{b_}{blk}{n + s}")
                        nc.scalar.activation(da[:], md[b_][blk][:], AF.Exp,
                                             scale=a_sb[:, blk,
                                                        n + s:n + s + 1])
                        das.append(da)
                        hs.append(pb.tile([128, L], BF16, tag="hn", bufs=4,
                                          name=f"h{b_}{blk}{n + s}"))
                    LQ = 512
                    for c in range(4):
                        sl = slice(c * LQ, (c + 1) * LQ)
                        for s in range(2):
                            init = 0.0 if c == 0 else hs[s][:, c * LQ - 1:c * LQ]
                            nc.vector.tensor_tensor_scan(
                                hs[s][:, sl], das[s][:, sl], u2[:, s, sl],
                                init, OP.mult, OP.add)
                            nc.tensor.matmul(
                                y_ps[c][:], identb_sb[:], hs[s][:, sl],
                                start=False, stop=(s == 1))
                        b_gate(b_, blk, y_ps, pts=[c])
                        out_proj(b_, mts=range(c * 4, c * 4 + 4), alt_tag=c)

                def b_gate(b_, blk, y_ps, pts=range(4)):
                    for pt in pts:
                        nc.vector.tensor_mul(
                            yin[b_][blk][:, pt * 512:(pt + 1) * 512],
                            y_ps[pt][:],
                            zac[b_][blk][:, pt * 512:(pt + 1) * 512])

                def b_block(b_, blk):
                    dtx, y_ps = b_prologue(b_, blk)
                    for i, n in enumerate(range(0, DS, 2)):
                        b_n_pair(b_, blk, n, dtx, y_ps=y_ps,
                                 pool=POOL_U2 and (i % 2 == 1))
                    b_gate(b_, blk, y_ps)
                    return y_ps

                def out_proj(b_, mts=None, alt_tag=None):
                    if mts is None:
                        mts = range(L // 128)
                    for mt in mts:
                        for dmh in range(2):
                            if alt_tag is not None and dmh == 1:
                                ps_o = pbps.tile([128, 512], F32,
                                                 tag=f"y_ps{alt_tag}")
                            else:
                                ps_o = paps.tile([128, 512], F32, tag="ps_in",
                                                 bufs=2)
                            for blk in range(NBLK):
                                nc.tensor.matmul(
                                    ps_o[:],
                                    yin[b_][blk][:, mt * 128:(mt + 1) * 128],
                                    wout_sb[:, blk, dmh * 512:(dmh + 1) * 512],
                                    start=(blk == 0), stop=(blk == NBLK - 1))
                            osb = pb.tile([128, 512], BF16, tag="osb", bufs=2)
                            nc.scalar.copy(osb[:], ps_o[:])
                            nc.sync.dma_start(
                                out_d[b_, mt * 128:(mt + 1) * 128,
                                      dmh * 512:(dmh + 1) * 512],
                                osb[:])

                # ---------------- schedule ----------------
                phase_a_chunk(0, 0, skip_z=True)
                phase_a_chunk(0, 1, skip_z=True)
                ar_ring1(0, 0)
                phase_a_chunk(0, 2, skip_z=True)
                phase_a_chunk(0, 3, skip_z=True)
                ar_ring1(0, 1)
                ar_ring2(0)
                dtin_load(0, 0)
                dt_half(0, 0)
                dtin_load(0, 1)
                dt_half(0, 1)
                y00 = mk_y_ps(0, 0)
                dtx00 = pb.tile([128, 1, L], BF16, tag="dtx", bufs=2,
                                name="dtx00")
                # pair 0 at half granularity; pairs 1..3 interleave with
                # phase A(1); ids deferred so PE never blocks on a scan
                hkeep = b_first_pairs(0, 0, dtx00)
                for ch in range(ncha):
                    for h in hkeep:
                        b_n_ids(y00, h, False)
                    hkeep = []
                    if ch < 2:
                        hkeep.extend(b_n_pair(0, 0, 2 * (ch + 2), dtx00))
                    phase_a_chunk(1, ch, skip_z=True)
                    if ch == 1:
                        ar_ring1(1, 0)
                    if ch == 3:
                        ar_ring1(1, 1)
                        ar_ring2(1)
                for h in hkeep:
                    b_n_ids(y00, h, False)
                for ch in range(ncha):
                    z_chunk(0, ch)
                for n in range(8, DS, 2):
                    b_n_pair(0, 0, n, dtx00, y_ps=y00)
                b_gate(0, 0, y00)
                # b(0,1) with z(1) interleaved: Act/PE have slack here,
                # keeping b1's z silus out of the congested phase-A window
                dtx01, y01 = b_prologue(0, 1)
                for i, n in enumerate(range(0, DS, 2)):
                    b_n_pair(0, 1, n, dtx01, y_ps=y01)
                    if 2 <= i < 6:
                        z_chunk(1, i - 2)
                b_gate(0, 1, y01)
                dtin_load(1, 0)
                dt_half(1, 0)
                dtin_load(1, 1)
                dt_half(1, 1)
                out_proj(0)
                b_block(1, 0)
                # b(1,1): last pair chunked so gates/out_proj overlap scans
                dtx11, y11 = b_prologue(1, 1)
                for n in range(0, DS - 2, 2):
                    b_n_pair(1, 1, n, dtx11, y_ps=y11)
                b_last_pair_chunked(1, 1, dtx11, y11)

    nc.compile()
    return nc


_NC_CACHE = {}


def _get_nc():
    if "nc" not in _NC_CACHE:
        _NC_CACHE["nc"] = build_nc()
    return _NC_CACHE["nc"]


def make_in_maps(x, W_in, conv_w, conv_b, W_x, W_dt, b_dt, A_log, D, W_out):
    x = np.asarray(x, np.float32)
    W_in = np.asarray(W_in, np.float32)
    conv_w = np.asarray(conv_w, np.float32)
    conv_b = np.asarray(conv_b, np.float32)
    W_x = np.asarray(W_x, np.float32)
    W_dt = np.asarray(W_dt, np.float32)
    b_dt = np.asarray(b_dt, np.float32)
    A_log = np.asarray(A_log, np.float32)
    D = np.asarray(D, np.float32)
    W_out = np.asarray(W_out, np.float32)

    xt = np.ascontiguousarray(
        x.reshape(B, L // 512, 512, KBLK, 128).transpose(0, 1, 4, 3, 2)
    ).astype(ml_dtypes.bfloat16)
    A = np.exp(A_log)

    in_maps = []
    for c in range(NCORES):
        lo = c * DIL
        sl = slice(lo, lo + DIL)
        cw = conv_w[sl]
        diagk = np.zeros((DC, DIL, 128), np.float32)
        for k in range(DC):
            for blk in range(NBLK):
                diagk[k, blk * 128:(blk + 1) * 128, :] = np.diag(
                    cw[blk * 128:(blk + 1) * 128, k])
        in_maps.append({
            "x_t": xt,
            "diagk": diagk.astype(ml_dtypes.bfloat16),
            "win": np.ascontiguousarray(
                np.concatenate([W_in[:, sl], W_in[:, DI + lo:DI + lo + DIL]],
                               axis=1)).astype(ml_dtypes.bfloat16),
            "wout": np.ascontiguousarray(W_out[sl]).astype(ml_dtypes.bfloat16),
            "wx": np.ascontiguousarray(
                np.concatenate([W_x[sl, :DS], -W_x[sl, DS:]], axis=1)
            ).astype(ml_dtypes.bfloat16),
            "wdt": np.ascontiguousarray(W_dt[:, sl]).astype(ml_dtypes.bfloat16),
            "a": np.ascontiguousarray(A[sl]),
            "convw": np.ascontiguousarray(conv_w[sl]),
            "convb": np.ascontiguousarray(conv_b[sl, None]),
            "dvec": np.ascontiguousarray(D[sl, None]),
            "bdt": np.ascontiguousarray(-b_dt[sl, None]),
            "identb": np.eye(128, dtype=ml_dtypes.bfloat16),
            "diagd": np.stack([np.diag(D[lo + k * 128:lo + (k + 1) * 128])
                               for k in range(NBLK)]).reshape(DIL, 128)
                       .astype(ml_dtypes.bfloat16),
        })
    return in_maps


def kernel(**inputs):
    nc = _get_nc()
    in_maps = make_in_maps(**inputs)
    res = run_bass_kernel_spmd(nc, in_maps, list(range(NCORES)))
    out = np.zeros((B, L, DM), np.float32)
    for c in range(NCORES):
        out += res.results[c]["out_p"].astype(np.float32)
    return out
